# revision 1
# baseline (speedup 1.0000x reference)
"""KDNet forward kernel for 8 Trainium2 NeuronCores.

Pure data parallelism per the sharding hint: the batch axis of x (512) is
sharded 64-per-core across the 8 cores; the tiny conv/fc weights and the
shared kd-tree index vectors c0..c10 are replicated. Each core runs the
11-level kd-conv pyramid + fc + log_softmax on its shard via one SPMD
(pmap) program; results are concatenated to the full [512, 16] output.
"""
import numpy as np
import jax
import jax.numpy as jnp
from functools import partial

DIMS = [2048, 1024, 512, 256, 128, 64, 32, 16, 8, 4, 2]
IN_CH = [3, 8, 32, 64, 64, 64, 128, 256, 512, 512, 512]
FEAT = [8, 32, 64, 64, 64, 128, 256, 512, 512, 512, 1024]
B = 512
NCORES = 8
K = 16

_COMPILED = {}


def _kdnet_shard(x, cs, Ws, bs, Wfc, bfc):
    """Forward for one batch shard. x: [64, 3, 2048]."""
    y = x
    for i in range(11):
        dim, f = DIMS[i], FEAT[i]
        W, b, sel = Ws[i], bs[i], cs[i]
        z = jnp.einsum('oi,bid->bod', W, y,
                       preferred_element_type=jnp.float32)
        z = jax.nn.relu(z + b[None, :, None])
        bsz = z.shape[0]
        z = z.reshape(bsz, f, 3 * dim)
        idx = sel + 3 * jnp.arange(dim, dtype=sel.dtype)
        z = jnp.take(z, idx, axis=2)
        z = z.reshape(bsz, f, dim // 2, 2)
        y = jnp.max(z, axis=-1)
    y = y.reshape(-1, 1024)
    logits = y @ Wfc.T + bfc
    return jax.nn.log_softmax(logits, axis=1)


def _get_compiled():
    if 'fn' not in _COMPILED:
        _COMPILED['fn'] = jax.pmap(
            _kdnet_shard,
            in_axes=(0, None, None, None, None, None),
            devices=jax.devices()[:NCORES],
        )
    return _COMPILED['fn']


def kernel(**inputs):
    x = np.asarray(inputs['x'], dtype=np.float32)
    cs = tuple(np.asarray(inputs[f'c{i}']).astype(np.int32) for i in range(11))
    Ws = tuple(np.asarray(inputs[f'W{i+1}'], dtype=np.float32) for i in range(11))
    bs = tuple(np.asarray(inputs[f'b{i+1}'], dtype=np.float32) for i in range(11))
    Wfc = np.asarray(inputs['Wfc'], dtype=np.float32)
    bfc = np.asarray(inputs['bfc'], dtype=np.float32)

    fn = _get_compiled()
    xs = x.reshape(NCORES, B // NCORES, 3, 2048)
    out = fn(xs, cs, Ws, bs, Wfc, bfc)
    out = np.asarray(out).reshape(B, K).astype(np.float32)
    return out


if __name__ == '__main__':
    rng = np.random.default_rng(0)
    inputs = {'x': rng.standard_normal((B, 3, 2048), dtype=np.float32)}
    for i, d in enumerate(DIMS):
        inputs[f'c{i}'] = rng.integers(0, 3, size=(d,)).astype(np.int64)
    for i in range(11):
        cin, f = IN_CH[i], FEAT[i]
        inputs[f'W{i+1}'] = (rng.standard_normal((3 * f, cin), dtype=np.float32)
                             / np.sqrt(cin))
        inputs[f'b{i+1}'] = np.zeros((3 * f,), dtype=np.float32)
    inputs['Wfc'] = rng.standard_normal((K, 1024), dtype=np.float32) / 32.0
    inputs['bfc'] = np.zeros((K,), dtype=np.float32)
    out = kernel(**inputs)
    print('out', out.shape, out.dtype, float(np.abs(out).max()))


# revision 3
# speedup vs baseline: 1.0108x; 1.0108x over previous
"""KDNet forward kernel for 8 Trainium2 NeuronCores.

Pure data parallelism per the sharding hint: the batch axis of x (512) is
sharded 64-per-core across the 8 cores; the tiny conv/fc weights and the
shared kd-tree index vectors c0..c10 are replicated. Each core runs the
11-level kd-conv pyramid + fc + log_softmax on its shard via one SPMD
(pmap) program; results are concatenated to the full [512, 16] output.
"""
import numpy as np
import jax
import jax.numpy as jnp
from functools import partial

DIMS = [2048, 1024, 512, 256, 128, 64, 32, 16, 8, 4, 2]
IN_CH = [3, 8, 32, 64, 64, 64, 128, 256, 512, 512, 512]
FEAT = [8, 32, 64, 64, 64, 128, 256, 512, 512, 512, 1024]
B = 512
NCORES = 8
K = 16

_COMPILED = {}


def _kdnet_shard(x, cs, Ws, bs, Wfc, bfc):
    """Forward for one batch shard. x: [64, 3, 2048]."""
    y = x
    for i in range(11):
        dim, f = DIMS[i], FEAT[i]
        W, b, sel = Ws[i], bs[i], cs[i]
        z = jnp.einsum('oi,bid->bod', W, y,
                       preferred_element_type=jnp.float32)
        z = jax.nn.relu(z + b[None, :, None])
        bsz = z.shape[0]
        z = z.reshape(bsz, f, 3 * dim)
        idx = sel + 3 * jnp.arange(dim, dtype=sel.dtype)
        z = jnp.take(z, idx, axis=2)
        z = z.reshape(bsz, f, dim // 2, 2)
        y = jnp.max(z, axis=-1)
    y = y.reshape(-1, 1024)
    logits = y @ Wfc.T + bfc
    return jax.nn.log_softmax(logits, axis=1)


def _get_compiled():
    if 'fn' not in _COMPILED:
        _COMPILED['fn'] = jax.pmap(
            _kdnet_shard,
            in_axes=(0, None, None, None, None, None),
            devices=jax.devices()[:NCORES],
        )
    return _COMPILED['fn']


def kernel(**inputs):
    x = np.asarray(inputs['x'], dtype=np.float32)
    cs = tuple(np.asarray(inputs[f'c{i}']).astype(np.int32) for i in range(11))
    Ws = tuple(np.asarray(inputs[f'W{i+1}'], dtype=np.float32) for i in range(11))
    bs = tuple(np.asarray(inputs[f'b{i+1}'], dtype=np.float32) for i in range(11))
    Wfc = np.asarray(inputs['Wfc'], dtype=np.float32)
    bfc = np.asarray(inputs['bfc'], dtype=np.float32)

    fn = _get_compiled()
    xs = x.reshape(NCORES, B // NCORES, 3, 2048)
    out = fn(xs, cs, Ws, bs, Wfc, bfc)
    out = np.asarray(out).reshape(B, K).astype(np.float32)
    return out


if __name__ == '__main__':
    rng = np.random.default_rng(0)
    inputs = {'x': rng.standard_normal((B, 3, 2048), dtype=np.float32)}
    for i, d in enumerate(DIMS):
        inputs[f'c{i}'] = rng.integers(0, 3, size=(d,)).astype(np.int64)
    for i in range(11):
        cin, f = IN_CH[i], FEAT[i]
        inputs[f'W{i+1}'] = (rng.standard_normal((3 * f, cin), dtype=np.float32)
                             / np.sqrt(cin))
        inputs[f'b{i+1}'] = np.zeros((3 * f,), dtype=np.float32)
    inputs['Wfc'] = rng.standard_normal((K, 1024), dtype=np.float32) / 32.0
    inputs['bfc'] = np.zeros((K,), dtype=np.float32)
    out = kernel(**inputs)
    print('out', out.shape, out.dtype, float(np.abs(out).max()))
